# revision 1
# baseline (speedup 1.0000x reference)
"""Trainium2 Bass kernel for nn_MAB_17471926960685 (dense_transformer).

Sharding: token-parallel over N. Each of 8 cores takes a 256-token slice of N
(both batches); attention keys are full (K/V computed replicated from Y).
No collectives.

Scores are computed transposed (keys on partitions, tokens on free axis)
against host-pretransposed bf16 encoding tables:
  - add_enc/16 is accumulated into the QK PSUM via an identity*(1/16) matmul
  - exp on ScalarE doubles as the PSUM->SBUF evacuation (bf16 out)
  - softmax denominators via ones-column matmuls on PE (sum over partitions)
  - mult_enc applied on VectorE in bf16 (2x mode)
  - 1/den broadcast onto MH_raw^T via selection-matrix matmuls
"""

import math
import sys

import numpy as np
import ml_dtypes

sys.path.insert(0, "/opt/trn_rl_repo")

import concourse.bass as bass
import concourse.mybir as mybir
import concourse.tile as tile
from concourse import bacc
from concourse.masks import make_identity
from concourse.bass_utils import run_bass_kernel_spmd

B, N, D, H = 2, 2048, 256, 8
DS = D // H          # 32
NCORES = 8
NL = N // NCORES     # 256 tokens per core per batch
TOK = B * NL         # 512 tokens per core
NKT = N // 128       # 16 key tiles
EPS = 1e-5
F32 = mybir.dt.float32
BF16 = mybir.dt.bfloat16
AX = mybir.AluOpType
AF = mybir.ActivationFunctionType


def _ln_apply(nc, pool, x_ap, g_bc, b_bc, out_ap):
    """LayerNorm rows of x_ap [128, D] -> out_ap (f32)."""
    stats = pool.tile([128, 6], F32, tag="ln_stats")
    mv = pool.tile([128, 2], F32, tag="ln_mv")
    nc.vector.bn_stats(out=stats, in_=x_ap)
    nc.vector.bn_aggr(out=mv, in_=stats)
    eps_t = pool.tile([128, 1], F32, tag="ln_eps")
    nc.vector.memset(eps_t, EPS)
    std = pool.tile([128, 1], F32, tag="ln_std")
    nc.scalar.activation(std, mv[:, 1:2], AF.Sqrt, bias=eps_t)
    rstd = pool.tile([128, 1], F32, tag="ln_rstd")
    nc.vector.reciprocal(rstd, std)
    xn = pool.tile([128, D], F32, tag="ln_xn")
    nc.vector.tensor_scalar(xn, x_ap, mv[:, 0:1], rstd, AX.subtract, AX.mult)
    nc.vector.tensor_tensor(xn, xn, g_bc, AX.mult)
    nc.vector.tensor_tensor(out_ap, xn, b_bc, AX.add)


def build_kernel(gelu_af=AF.Gelu_apprx_tanh):
    nc = bacc.Bacc()
    P = {}
    for name, shape in [
        ("Xs", [B, NL, D]),
        ("bq", [D]), ("bk", [D]), ("bv", [D]), ("bmix", [D]),
        ("g0", [D]), ("b0", [D]), ("g1", [D]), ("b1", [D]),
    ]:
        P[name] = nc.declare_dram_parameter(name, shape, F32, isOutput=False)
    for name, shape in [
        ("Y", [B, N, D]),
        ("Wq", [D, D]), ("Wk", [D, D]), ("Wv", [D, D]), ("Wmix", [D, D]),
        ("wi0", [4 * D, D]), ("wi1", [4 * D, D]), ("wo", [D, 4 * D]),
        ("addT", [H, N, NL]), ("multT", [H, N, NL]),
    ]:
        P[name] = nc.declare_dram_parameter(name, shape, BF16, isOutput=False)
    out_ext = nc.declare_dram_parameter("out", [B, NL, D], F32, isOutput=True)

    with tile.TileContext(nc) as tc:
        with tc.tile_pool(name="persist", bufs=1) as pp, \
             tc.tile_pool(name="wload", bufs=2) as wlp, \
             tc.tile_pool(name="ln", bufs=2) as lnp, \
             tc.tile_pool(name="enc", bufs=2) as encp, \
             tc.tile_pool(name="pa", bufs=2) as pap, \
             tc.tile_pool(name="ytp", bufs=1) as ytp, \
             tc.tile_pool(name="psA", bufs=2, space="PSUM") as psA, \
             tc.tile_pool(name="psB", bufs=2, space="PSUM") as psB, \
             tc.tile_pool(name="psS", bufs=2, space="PSUM") as psS, \
             tc.tile_pool(name="psM", bufs=1, space="PSUM") as psM, \
             tc.tile_pool(name="psD", bufs=1, space="PSUM") as psD:

            # ---------- constants ----------
            id16 = pp.tile([128, 128], BF16)
            make_identity(nc, id16)
            nc.vector.tensor_scalar_mul(id16, id16, 1.0 / 16.0)
            ones_col = pp.tile([128, 1], BF16)
            nc.vector.memset(ones_col, 1.0)
            ones_row = pp.tile([1, TOK], F32)
            nc.vector.memset(ones_row, 1.0)

            brow = {}
            for name in ("bq", "bk", "bv"):
                t = pp.tile([1, D], F32, tag=f"brow_{name}")
                nc.sync.dma_start(out=t,
                                  in_=P[name][:].rearrange("(o d) -> o d", o=1))
                brow[name] = t
            bcast = {}
            for name in ("g0", "b0", "g1", "b1", "bmix"):
                t = pp.tile([128, D], F32, tag=f"bc_{name}")
                ap = P[name][:].rearrange("(o d) -> o d", o=1)
                bap = bass.AP(tensor=ap.tensor, offset=ap.offset,
                              ap=[[0, 128], ap.ap[1]])
                nc.sync.dma_start(out=t, in_=bap)
                bcast[name] = t
            mask_all = pp.tile([1, 4 * 128], BF16)
            nc.vector.memset(mask_all, 0.0)
            for j in range(4):
                nc.vector.memset(mask_all[0:1, j * 128 + 32 * j:
                                          j * 128 + 32 * j + 32], 1.0)

            # ---------- weights: load + PE-transpose -> W^T bf16 ----------
            def load_wT(hnd, rows, cols, tagp):
                """DRAM [rows, cols] -> W^T bf16 tiles: cols//128 tiles of
                [128 (col block), rows]."""
                tiles = [pp.tile([128, rows], BF16, tag=f"{tagp}{i}", name=f"{tagp}{i}") for i in range(cols // 128)]
                for ri in range(rows // 128):
                    w_n = wlp.tile([128, cols], BF16, tag="wstage")
                    nc.sync.dma_start(
                        out=w_n,
                        in_=hnd[:].rearrange("(t p) c -> t p c", p=128)[ri])
                    for co in range(cols // 128):
                        nc.sync.dma_start(
                            out=tiles[co][:, ri * 128:(ri + 1) * 128],
                            in_=w_n[:, co * 128:(co + 1) * 128],
                            transpose=True)
                return tiles

            WqT = load_wT(P["Wq"], D, D, "WqT")        # 2 x [128(dq), 256(de)]
            WkT = load_wT(P["Wk"], D, D, "WkT")
            WvT = load_wT(P["Wv"], D, D, "WvT")
            WmixT = load_wT(P["Wmix"], D, D, "WmixT")
            wi0T = load_wT(P["wi0"], 4 * D, D, "wi0T")  # 2 x [128(do), 1024(u)]
            wi1T = load_wT(P["wi1"], 4 * D, D, "wi1T")
            woT = load_wT(P["wo"], D, 4 * D, "woT")    # 8 x [128(u), 256(do)]

            # ---------- phase 1: LN0(X rows); Q^T (scores) and Q_N (residual) --
            lnx_n = []
            for b in range(B):
                x_n = wlp.tile([128, 2 * D], F32, tag="xload")
                nc.sync.dma_start(
                    out=x_n.rearrange("p (s d) -> p s d", s=2),
                    in_=P["Xs"][b].rearrange("(s p) d -> p s d", p=128))
                for s in range(2):
                    o = pp.tile([128, D], F32, tag=f"lnx{b}{s}")
                    _ln_apply(nc, lnp, x_n[:, s * D:(s + 1) * D],
                              bcast["g0"], bcast["b0"], o)
                    lnx_n.append(o)                      # tt = b*2 + s
            lnxT = [pp.tile([128, TOK], BF16, tag=f"lnxT{i}", name=f"lnxT{i}") for i in range(2)]
            for tt in range(4):
                lnxb = pap.tile([128, D], BF16, tag="lnxb")
                nc.scalar.copy(lnxb, lnx_n[tt])
                for dq in range(2):
                    nc.sync.dma_start(
                        out=lnxT[dq][:, tt * 128:(tt + 1) * 128],
                        in_=lnxb[:, dq * 128:(dq + 1) * 128], transpose=True)

            # Q^T/16 bf16: 4 tiles [64, TOK] (2 heads each at bases 0/32)
            qsT = [pp.tile([64, TOK], BF16, tag=f"qsT{i}", name=f"qsT{i}") for i in range(4)]
            for j in range(4):
                ps = psB.tile([64, TOK], F32, tag="big")
                for kq in range(2):
                    nc.tensor.matmul(ps, WqT[kq][:, j * 64:(j + 1) * 64],
                                     lnxT[kq], start=(kq == 0), stop=False)
                nc.tensor.matmul(ps, brow["bq"][0:1, j * 64:(j + 1) * 64],
                                 ones_row, start=False, stop=True)
                nc.scalar.activation(qsT[j], ps, AF.Copy, scale=1.0 / 16.0)
            # Q_N f32 (residual, includes bq): out[tok block, de]
            qN = []
            for tt in range(4):
                ps = psB.tile([128, D], F32, tag="big")
                for kq in range(2):
                    nc.tensor.matmul(ps, lnxT[kq][:, tt * 128:(tt + 1) * 128],
                                     WqT[kq], start=(kq == 0), stop=False)
                nc.tensor.matmul(ps, ones_row[0:1, 0:128], brow["bq"],
                                 start=False, stop=True)
                t = pp.tile([128, D], F32, tag=f"qN{tt}")
                nc.scalar.copy(t, ps)
                qN.append(t)

            # ---------- phase 2: Y^T; K^T bf16; V_N bf16 ----------
            kT = []   # [b][de block] -> [128, N] bf16
            vN = []   # [b] -> [128, NKT*256] bf16 (key block kt at cols kt*256)
            for b in range(B):
                yT = [ytp.tile([128, N], BF16, tag=f"yT{i}", name=f"yT{i}") for i in range(2)]
                yn = ytp.tile([128, NKT * D], BF16, tag="yn")
                nc.sync.dma_start(
                    out=yn.rearrange("p (nt d) -> p nt d", nt=NKT),
                    in_=P["Y"][b].rearrange("(nt p) d -> p nt d", p=128))
                for nt in range(NKT):
                    for dd in range(2):
                        nc.sync.dma_start(
                            out=yT[dd][:, nt * 128:(nt + 1) * 128],
                            in_=yn[:, nt * D + dd * 128:nt * D + (dd + 1) * 128],
                            transpose=True)
                ktb = []
                for j in range(4):
                    t = pp.tile([64, N], BF16, tag=f"kT{b}{j}", name=f"kT{b}{j}")
                    for ch in range(N // 512):
                        ps = psB.tile([64, 512], F32, tag="big")
                        sl = slice(ch * 512, (ch + 1) * 512)
                        for kd in range(2):
                            nc.tensor.matmul(
                                ps, WkT[kd][:, j * 64:(j + 1) * 64],
                                yT[kd][:, sl], start=(kd == 0), stop=False)
                        nc.tensor.matmul(
                            ps, brow["bk"][0:1, j * 64:(j + 1) * 64],
                            ones_row[0:1, 0:512], start=False, stop=True)
                        nc.scalar.copy(t[:, sl], ps)
                    ktb.append(t)
                kT.append(ktb)
                vb = pp.tile([128, NKT * D], BF16, tag=f"vN{b}")
                for kt in range(NKT):
                    ps = psB.tile([128, D], F32, tag="big")
                    for kd in range(2):
                        nc.tensor.matmul(
                            ps, yT[kd][:, kt * 128:(kt + 1) * 128], WvT[kd],
                            start=(kd == 0), stop=False)
                    nc.tensor.matmul(ps, ones_row[0:1, 0:128], brow["bv"],
                                     start=False, stop=True)
                    nc.scalar.copy(vb[:, kt * D:(kt + 1) * D], ps)
                vN.append(vb)

            # ---------- phase 3: attention ----------
            recip_wide = pp.tile([1, 16 * NL], BF16)
            mhT = [pp.tile([128, TOK], BF16, tag=f"mhT{i}", name=f"mhT{i}") for i in range(2)]
            for h in range(H):
                at_h = encp.tile([128, NKT * NL], BF16, tag="addT")
                nc.sync.dma_start(
                    out=at_h.rearrange("p (kt t) -> p kt t", kt=NKT),
                    in_=P["addT"][h].rearrange("(kt p) t -> p kt t", p=128))
                mt_h = encp.tile([128, NKT * NL], BF16, tag="multT")
                nc.sync.dma_start(
                    out=mt_h.rearrange("p (kt t) -> p kt t", kt=NKT),
                    in_=P["multT"][h].rearrange("(kt p) t -> p kt t", p=128))
                g, r = h // 4, 32 * (h % 4)
                j, r2 = h // 2, 32 * (h % 2)
                for b in range(B):
                    ps_mh = psM.tile([32, NL], F32, tag="mh")
                    ps_den = psD.tile([1, NL], F32, tag="den")
                    for kt in range(NKT):
                        ps_s = psS.tile([128, NL], F32, tag="s")
                        nc.tensor.matmul(
                            ps_s,
                            kT[b][j][r2:r2 + DS, kt * 128:(kt + 1) * 128],
                            qsT[j][r2:r2 + DS, b * NL:(b + 1) * NL],
                            start=True, stop=False)
                        nc.tensor.matmul(
                            ps_s, id16, at_h[:, kt * NL:(kt + 1) * NL],
                            start=False, stop=True)
                        pt = pap.tile([128, NL], BF16, tag="pt")
                        nc.scalar.activation(pt, ps_s, AF.Exp)
                        nc.tensor.matmul(ps_den, ones_col, pt,
                                         start=(kt == 0), stop=(kt == NKT - 1))
                        at = pap.tile([128, NL], BF16, tag="at")
                        nc.vector.tensor_tensor(
                            at, pt, mt_h[:, kt * NL:(kt + 1) * NL], AX.mult)
                        nc.tensor.matmul(
                            ps_mh,
                            vN[b][:, kt * D + r + 128 * g:
                                  kt * D + r + 128 * g + DS],
                            at, start=(kt == 0), stop=(kt == NKT - 1))
                    q = b * 8 + h
                    rcp = lnp.tile([1, NL], F32, tag="rcp")
                    nc.vector.reciprocal(rcp, ps_den)
                    nc.vector.tensor_copy(
                        recip_wide[0:1, q * NL:(q + 1) * NL], rcp)
                    nc.scalar.copy(mhT[g][r:r + DS, b * NL:(b + 1) * NL], ps_mh)

            # ---------- phase 4: 1/den, mix, residual ----------
            rb = [pp.tile([128, TOK], BF16, tag=f"rb{i}", name=f"rb{i}") for i in range(2)]
            for t in range(2):
                for b in range(B):
                    ps = psA.tile([128, NL], F32, tag="sm")
                    for hh in range(4):
                        q = b * 8 + 4 * t + hh
                        nc.tensor.matmul(
                            ps, mask_all[0:1, hh * 128:(hh + 1) * 128],
                            recip_wide[0:1, q * NL:(q + 1) * NL],
                            start=(hh == 0), stop=(hh == 3))
                    nc.scalar.copy(rb[t][:, b * NL:(b + 1) * NL], ps)
            mhsT = [pp.tile([128, TOK], BF16, tag=f"mhsT{i}", name=f"mhsT{i}") for i in range(2)]
            for t in range(2):
                nc.vector.tensor_tensor(mhsT[t], mhT[t], rb[t], AX.mult)
            mxT = [pp.tile([128, TOK], BF16, tag=f"mxT{i}", name=f"mxT{i}") for i in range(2)]
            for t in range(2):
                ps = psB.tile([128, TOK], F32, tag="big")
                for kd in range(2):
                    nc.tensor.matmul(ps, WmixT[kd][:, t * 128:(t + 1) * 128],
                                     mhsT[kd], start=(kd == 0),
                                     stop=(kd == 1))
                nc.scalar.copy(mxT[t], ps)
            hid = []
            for tt in range(4):
                t = pp.tile([128, D], F32, tag=f"hid{tt}")
                hid.append(t)
            for tt in range(4):
                for t in range(2):
                    mixn = pap.tile([128, 128], BF16, tag="mixn")
                    nc.sync.dma_start(out=mixn,
                                      in_=mxT[t][:, tt * 128:(tt + 1) * 128],
                                      transpose=True)
                    sl = slice(t * 128, (t + 1) * 128)
                    nc.vector.tensor_tensor(hid[tt][:, sl], mixn, qN[tt][:, sl],
                                            AX.add)
                    nc.vector.tensor_tensor(hid[tt][:, sl], hid[tt][:, sl],
                                            bcast["bmix"][:, sl], AX.add)

            # ---------- phase 5: LN1 + FFN + residual out ----------
            hrT = [pp.tile([128, TOK], BF16, tag=f"hrT{i}", name=f"hrT{i}") for i in range(2)]
            for tt in range(4):
                hr = lnp.tile([128, D], F32, tag="hr")
                _ln_apply(nc, lnp, hid[tt], bcast["g1"], bcast["b1"], hr)
                hrb = pap.tile([128, D], BF16, tag="hrb")
                nc.scalar.copy(hrb, hr)
                for dd in range(2):
                    nc.sync.dma_start(
                        out=hrT[dd][:, tt * 128:(tt + 1) * 128],
                        in_=hrb[:, dd * 128:(dd + 1) * 128], transpose=True)
            ffin = []
            for m in range(8):
                ps0 = psB.tile([128, TOK], F32, tag="big")
                ps1 = psB.tile([128, TOK], F32, tag="big")
                for kd in range(2):
                    nc.tensor.matmul(ps0, wi0T[kd][:, m * 128:(m + 1) * 128],
                                     hrT[kd], start=(kd == 0), stop=(kd == 1))
                    nc.tensor.matmul(ps1, wi1T[kd][:, m * 128:(m + 1) * 128],
                                     hrT[kd], start=(kd == 0), stop=(kd == 1))
                gt = pap.tile([128, TOK], BF16, tag="gelu")
                nc.scalar.activation(gt, ps0, gelu_af)
                ut = pap.tile([128, TOK], BF16, tag="u1c")
                nc.scalar.copy(ut, ps1)
                ft = pp.tile([128, TOK], BF16, tag=f"ffin{m}")
                nc.vector.tensor_tensor(ft, gt, ut, AX.mult)
                ffin.append(ft)
            for t in range(2):
                ps = psB.tile([128, TOK], F32, tag="big")
                for ku in range(8):
                    nc.tensor.matmul(ps, woT[ku][:, t * 128:(t + 1) * 128],
                                     ffin[ku], start=(ku == 0), stop=(ku == 7))
                fft = pap.tile([128, TOK], BF16, tag="ffT")
                nc.scalar.copy(fft, ps)
                for tt in range(4):
                    ffn = pap.tile([128, 128], BF16, tag="ffn")
                    nc.sync.dma_start(out=ffn,
                                      in_=fft[:, tt * 128:(tt + 1) * 128],
                                      transpose=True)
                    o = pap.tile([128, 128], F32, tag="outN")
                    nc.vector.tensor_tensor(
                        o, ffn, hid[tt][:, t * 128:(t + 1) * 128], AX.add)
                    nc.sync.dma_start(
                        out=out_ext[tt // 2].rearrange(
                            "(s p) d -> s p d", p=128)[tt % 2][:, t * 128:(t + 1) * 128],
                        in_=o)
    nc.finalize()
    return nc


_SEL = None


def _selmask_np():
    global _SEL
    if _SEL is None:
        s = np.zeros((16, 2 * B * 128), np.float32)
        for t in range(2):
            for b in range(B):
                for p in range(128):
                    s[b * 8 + t * 4 + p // 32, (t * B + b) * 128 + p] = 1.0
        _SEL = s
    return _SEL


def prepare_in_maps(inputs):
    bf = ml_dtypes.bfloat16
    X = np.asarray(inputs["X"], np.float32)
    Yf = np.asarray(inputs["Y"], np.float32)
    add_enc = np.asarray(inputs["add_enc"], np.float32)
    mult_enc = np.asarray(inputs["mult_enc"], np.float32)
    common = {k: np.asarray(inputs[k], np.float32)
              for k in ("bq", "bk", "bv", "bmix", "g0", "b0", "g1", "b1")}
    for k in ("Wq", "Wk", "Wv", "Wmix", "wi0", "wi1", "wo"):
        common[k] = np.asarray(inputs[k], np.float32).astype(bf)
    common["Y"] = Yf.astype(bf)
    in_maps = []
    for c in range(NCORES):
        sl = slice(c * NL, (c + 1) * NL)
        m = dict(common)
        m["Xs"] = np.ascontiguousarray(X[:, sl, :])
        m["addT"] = np.ascontiguousarray(
            add_enc[:, sl, :].transpose(0, 2, 1)).astype(bf)
        m["multT"] = np.ascontiguousarray(
            mult_enc[:, sl, :].transpose(0, 2, 1)).astype(bf)
        in_maps.append(m)
    return in_maps


def kernel(**inputs):
    in_maps = prepare_in_maps(inputs)
    nc = build_kernel()
    res = run_bass_kernel_spmd(nc, in_maps, list(range(NCORES)))
    out = np.empty((B, N, D), np.float32)
    for c in range(NCORES):
        out[:, c * NL:(c + 1) * NL, :] = res.results[c]["out"]
    return out


if __name__ == "__main__":
    nc = build_kernel()
    print("build OK")



# revision 20
# speedup vs baseline: 1.9384x; 1.9384x over previous
"""Trainium2 Bass kernel for nn_MAB_17471926960685 (dense_transformer).

Token-parallel over N: each core takes a 256-token query slice (both
batches); K/V computed for full N from host-pretransposed Y^T.

Attention dataflow (scores^T: keys on partitions, queries free):
  - Tables host-folded: A' = add_enc + 16*ln(mult_enc)  (numerator table)
                        R  = 1/mult_enc                 (denominator fixup)
    with the reference's head mapping (head h, batch b uses table
    (2h+b)%8; blocks j=0..3 serve heads j and j+4).
  - QK: 4 heads row-packed via tile_position=(32j,0), K=32 each.
  - A' accumulated into score PSUM via identity matmul.
  - One exp (scale=1/16) evacuates [128,1024] PSUM -> P' bf16 = numerator.
  - P = P' * R on VectorE (bf16 2x) = plain exp for the denominator.
  - den: ones-column matmuls col-packed via tile_position=(0,32j).
  - MH: V-slice matmuls col-packed via tile_position=(0,32j).
"""

import math
import sys

import numpy as np
import ml_dtypes

sys.path.insert(0, "/opt/trn_rl_repo")

import concourse.bass as bass
import concourse.mybir as mybir
import concourse.tile as tile
from concourse import bacc
from concourse.masks import make_identity
from concourse.bass_utils import run_bass_kernel_spmd

B, N, D, H = 2, 2048, 256, 8
DS = D // H          # 32
NCORES = 8
NL = N // NCORES     # 256 queries per core per batch
TOK = B * NL         # 512
NKT = N // 128       # 16 key tiles
D4 = 4 * D           # 1024
EPS = 1e-5
F32 = mybir.dt.float32
BF16 = mybir.dt.bfloat16
AX = mybir.AluOpType
AF = mybir.ActivationFunctionType


def _ln_apply(nc, pool, x_ap, g_bc, b_bc, out_ap):
    """LayerNorm rows of x_ap [128, D] -> out_ap (f32)."""
    stats = pool.tile([128, 6], F32, tag="ln_stats")
    mv = pool.tile([128, 2], F32, tag="ln_mv")
    nc.vector.bn_stats(out=stats, in_=x_ap)
    nc.vector.bn_aggr(out=mv, in_=stats)
    eps_t = pool.tile([128, 1], F32, tag="ln_eps")
    nc.vector.memset(eps_t, EPS)
    std = pool.tile([128, 1], F32, tag="ln_std")
    nc.scalar.activation(std, mv[:, 1:2], AF.Sqrt, bias=eps_t)
    rstd = pool.tile([128, 1], F32, tag="ln_rstd")
    nc.vector.reciprocal(rstd, std)
    xn = pool.tile([128, D], F32, tag="ln_xn")
    nc.vector.tensor_scalar(xn, x_ap, mv[:, 0:1], rstd, AX.subtract, AX.mult)
    nc.vector.tensor_tensor(xn, xn, g_bc, AX.mult)
    nc.vector.tensor_tensor(out_ap, xn, b_bc, AX.add)


def build_kernel(gelu_af=AF.Gelu_apprx_tanh):
    nc = bacc.Bacc()
    P = {}
    for name, shape, dt in [
        ("Xs", [B, NL, D], F32),
        ("bq", [D], F32), ("bv", [D], F32), ("bmix", [D], F32),
        ("g0", [D], F32), ("b0", [D], F32), ("g1", [D], F32), ("b1", [D], F32),
        ("Yt", [B, 2, 128, N], BF16),
        ("WqT", [2, 128, D], BF16), ("WkT", [2, 128, D], BF16),
        ("WvT", [2, 128, D], BF16), ("WmixT", [2, 128, D], BF16),
        ("wi0T", [2, 128, D4], BF16), ("wi1T", [2, 128, D4], BF16),
        ("woT", [8, 128, D], BF16),
        ("addT", [B, NKT, 128, 4 * NL], BF16),
        ("multR", [B, NKT, 128, 4 * NL], BF16),
    ]:
        P[name] = nc.declare_dram_parameter(name, shape, dt, isOutput=False)
    out_ext = nc.declare_dram_parameter("out", [B, NL, D], F32, isOutput=True)

    with tile.TileContext(nc) as tc:
        with tc.tile_pool(name="pp", bufs=1) as pp, \
             tc.tile_pool(name="enc", bufs=3) as encp, \
             tc.tile_pool(name="pex", bufs=3) as pxp, \
             tc.tile_pool(name="ln", bufs=2) as lnp, \
             tc.tile_pool(name="wk", bufs=2) as wkp:

            # ---------- constants ----------
            idb = pp.tile([128, 128], BF16)
            make_identity(nc, idb)
            zstat = pp.tile([128, 128], BF16)
            nc.vector.memset(zstat, 0.0)
            zdum = pp.tile([128, 512], BF16)
            nc.vector.memset(zdum, 0.0)
            ones_blk = pp.tile([128, 32], BF16)
            nc.vector.memset(ones_blk, 0.0)
            nc.vector.memset(ones_blk[:, 0:1], 1.0)
            ones_row = pp.tile([1, TOK], F32)
            nc.vector.memset(ones_row, 1.0)
            # maskt row 32j: ones at cols [32j, 32j+32) — rb broadcast lhsT
            maskt = pp.tile([128, 128], F32)
            nc.vector.memset(maskt, 0.0)
            for j in range(4):
                nc.vector.memset(maskt[32 * j:32 * j + 1,
                                       32 * j:32 * j + 32], 1.0)
            # gfill: 1.0 on non-denominator rows (keeps 1/dn finite there)
            gfill = pp.tile([1, 128], F32)
            nc.vector.memset(gfill, 1.0)
            for j in range(4):
                nc.vector.memset(gfill[0:1, 32 * j:32 * j + 1], 0.0)
            brow_bq = pp.tile([1, D], F32)
            nc.sync.dma_start(out=brow_bq,
                              in_=P["bq"][:].rearrange("(o d) -> o d", o=1))
            bcast = {}
            for nm in ("g0", "b0", "g1", "b1", "bmix", "bv"):
                t = pp.tile([128, D], F32, tag=f"bc_{nm}", name=f"bc_{nm}")
                ap = P[nm][:].rearrange("(o d) -> o d", o=1)
                bap = bass.AP(tensor=ap.tensor, offset=ap.offset,
                              ap=[[0, 128], ap.ap[1]])
                nc.sync.dma_start(out=t, in_=bap)
                bcast[nm] = t

            # ---------- weights (host-pretransposed, plain loads) ----------
            def loadw(hnd, nchunk, width, nm):
                t = pp.tile([128, nchunk * width], BF16, tag=nm, name=nm)
                nc.sync.dma_start(
                    out=t.rearrange("p (c d) -> p c d", c=nchunk),
                    in_=hnd[:].rearrange("c p d -> p c d"))
                return t
            wqTt = loadw(P["WqT"], 2, D, "wqTt")
            wkTt = loadw(P["WkT"], 2, D, "wkTt")
            wvTt = loadw(P["WvT"], 2, D, "wvTt")
            wmixTt = loadw(P["WmixT"], 2, D, "wmixTt")
            wi0Tt = loadw(P["wi0T"], 2, D4, "wi0Tt")
            wi1Tt = loadw(P["wi1T"], 2, D4, "wi1Tt")
            woTt = loadw(P["woT"], 8, D, "woTt")
            ytb = []
            for b in range(B):
                t = pp.tile([128, 2 * N], BF16, tag=f"yt{b}", name=f"yt{b}")
                nc.sync.dma_start(
                    out=t.rearrange("p (c n) -> p c n", c=2),
                    in_=P["Yt"][b].rearrange("c p n -> p c n"))
                ytb.append(t)

            # ---------- phase 1: LN0, Xn^T, Q^T, Q_N ----------
            XnT = pp.tile([128, 2 * TOK], BF16)
            with tc.tile_pool(name="psB", bufs=2, space="PSUM") as psB:
                for b in range(B):
                    x_n = wkp.tile([128, 2 * D], F32, tag="xload")
                    nc.sync.dma_start(
                        out=x_n.rearrange("p (s d) -> p s d", s=2),
                        in_=P["Xs"][b].rearrange("(s p) d -> p s d", p=128))
                    for s in range(2):
                        xo = lnp.tile([128, D], F32, tag="xn")
                        _ln_apply(nc, lnp, x_n[:, s * D:(s + 1) * D],
                                  bcast["g0"], bcast["b0"], xo)
                        xb = wkp.tile([128, D], BF16, tag="xnb")
                        nc.scalar.copy(xb, xo)
                        tt = b * 2 + s
                        for c in range(2):
                            nc.sync.dma_start(
                                out=XnT[:, TOK * c + 128 * tt:
                                        TOK * c + 128 * tt + 128],
                                in_=xb[:, 128 * c:128 * c + 128],
                                transpose=True)
                qTq = [pp.tile([128, TOK], BF16, tag=f"qT{qg}", name=f"qT{qg}")
                       for qg in range(2)]
                for qg in range(2):
                    ps = psB.tile([128, 512], F32, tag="big")
                    for c in range(2):
                        nc.tensor.matmul(
                            ps, wqTt[:, D * c + 128 * qg:D * c + 128 * qg + 128],
                            XnT[:, TOK * c:TOK * (c + 1)],
                            start=(c == 0), stop=False)
                    nc.tensor.matmul(ps, brow_bq[0:1, 128 * qg:128 * qg + 128],
                                     ones_row, start=False, stop=True)
                    nc.vector.tensor_copy(qTq[qg], ps)
                qN = [pp.tile([128, D], F32, tag=f"qN{tt}", name=f"qN{tt}")
                      for tt in range(4)]
                for tt in range(4):
                    ps = psB.tile([128, 512], F32, tag="big")
                    for c in range(2):
                        nc.tensor.matmul(
                            ps[:, 0:D],
                            XnT[:, TOK * c + 128 * tt:TOK * c + 128 * tt + 128],
                            wqTt[:, D * c:D * (c + 1)],
                            start=(c == 0), stop=False)
                    nc.tensor.matmul(ps[:, 0:D], ones_row[0:1, 0:128], brow_bq,
                                     start=False, stop=True)
                    # fold bmix into the residual now
                    nc.vector.tensor_tensor(qN[tt], ps[:, 0:D], bcast["bmix"],
                                            AX.add)

                # ---------- phase 2: K^T (quad-major), V_N ----------
                kTq = [pp.tile([128, N], BF16, tag=f"kT{i}", name=f"kT{i}")
                       for i in range(4)]      # index b*2+qg
                for b in range(B):
                    for qg in range(2):
                        for ck in range(4):
                            ps = psB.tile([128, 512], F32, tag="big")
                            for c in range(2):
                                nc.tensor.matmul(
                                    ps,
                                    wkTt[:, D * c + 128 * qg:
                                         D * c + 128 * qg + 128],
                                    ytb[b][:, N * c + 512 * ck:
                                           N * c + 512 * (ck + 1)],
                                    start=(c == 0), stop=(c == 1))
                            nc.vector.tensor_copy(
                                kTq[b * 2 + qg][:, 512 * ck:512 * (ck + 1)], ps)
                vN = [pp.tile([128, NKT * D], BF16, tag=f"vN{b}", name=f"vN{b}")
                      for b in range(B)]
                for b in range(B):
                    for kt in range(NKT):
                        ps = psB.tile([128, 512], F32, tag="big")
                        for c in range(2):
                            nc.tensor.matmul(
                                ps[:, 0:D],
                                ytb[b][:, N * c + 128 * kt:N * c + 128 * kt + 128],
                                wvTt[:, D * c:D * (c + 1)],
                                start=(c == 0), stop=(c == 1))
                        nc.vector.tensor_tensor(
                            vN[b][:, D * kt:D * (kt + 1)], ps[:, 0:D],
                            bcast["bv"], AX.add)

            # ---------- phase 3: attention ----------
            mhsT = [pp.tile([128, TOK], BF16, tag=f"mhsT{qg}", name=f"mhsT{qg}")
                    for qg in range(2)]
            with tc.tile_pool(name="psS", bufs=1, space="PSUM") as psS, \
                 tc.tile_pool(name="psM", bufs=2, space="PSUM") as psM, \
                 tc.tile_pool(name="psD", bufs=2, space="PSUM") as psD:
                # scoreT: 4 banks; head j owns bank j (cols 512j..), qg
                # parity picks the 256-col half (double buffer). Concurrent
                # row-packed QK matmuls thus never share a PSUM bank.
                scoreT = psS.tile([128, 2048], F32)
                sc3 = scoreT.rearrange("pp (j c) -> pp j c", j=4)
                for b in range(B):
                    mh = psM.tile([128, 512], F32, tag="mh")
                    dn = psD.tile([128, 512], F32, tag="dn")
                    # zero-fill once (start=True writes full bank, sets
                    # has_written everywhere) so col-packed accumulation
                    # below can use start=False throughout.
                    nc.tensor.matmul(mh, zstat, zdum, start=True, stop=False,
                                     skip_group_check=True)
                    nc.tensor.matmul(dn, zstat, zdum, start=True, stop=False,
                                     skip_group_check=True)
                    nc.tensor.matmul(dn, gfill, ones_row[0:1, 0:512],
                                     start=False, stop=False,
                                     skip_group_check=True)
                    for kt in range(NKT):
                        addc = encp.tile([128, 4 * NL], BF16, tag="addc")
                        nc.sync.dma_start(out=addc, in_=P["addT"][b][kt])
                        rc = encp.tile([128, 4 * NL], BF16, tag="rc")
                        nc.sync.dma_start(out=rc, in_=P["multR"][b][kt])
                        for qg in range(2):
                            # A' tables first: start=True writes the head's
                            # region (sets has_written), QK then accumulates.
                            for j in range(4):
                                nc.tensor.matmul(
                                    scoreT[:, 512 * j + 256 * qg:
                                           512 * j + 256 * (qg + 1)], idb,
                                    addc[:, 256 * j:256 * (j + 1)],
                                    start=True, stop=False,
                                    skip_group_check=True)
                            for j in range(4):
                                nc.tensor.matmul(
                                    scoreT[:, 512 * j + 256 * qg:
                                           512 * j + 256 * (qg + 1)],
                                    kTq[b * 2 + qg][32 * j:32 * j + 32,
                                                    128 * kt:128 * kt + 128],
                                    qTq[qg][32 * j:32 * j + 32,
                                            NL * b:NL * (b + 1)],
                                    start=False, stop=True,
                                    tile_position=(32 * j, 0),
                                    skip_group_check=True)
                            pe = pxp.tile([128, 1024], BF16, tag="pe")
                            nc.scalar.activation(
                                pe.rearrange("pp (j c) -> pp j c", j=4),
                                sc3[:, :, 256 * qg:256 * (qg + 1)],
                                AF.Exp, scale=1.0 / 16.0)
                            pd = pxp.tile([128, 1024], BF16, tag="pd")
                            nc.vector.tensor_tensor(pd, pe, rc, AX.mult)
                            for j in range(4):
                                nc.tensor.matmul(
                                    dn[32 * j:32 * j + 32, NL * qg:NL * (qg + 1)],
                                    ones_blk, pd[:, 256 * j:256 * (j + 1)],
                                    start=False,
                                    stop=(kt == NKT - 1 and qg == 1),
                                    tile_position=(0, 32 * j),
                                    skip_group_check=True)
                            for j in range(4):
                                nc.tensor.matmul(
                                    mh[32 * j:32 * j + 32, NL * qg:NL * (qg + 1)],
                                    vN[b][:, D * kt + 128 * qg + 32 * j:
                                          D * kt + 128 * qg + 32 * j + 32],
                                    pe[:, 256 * j:256 * (j + 1)],
                                    start=False,
                                    stop=(kt == NKT - 1 and qg == 1),
                                    tile_position=(0, 32 * j),
                                    skip_group_check=True)
                    rcpt = wkp.tile([128, 512], F32, tag="rcpt")
                    nc.vector.reciprocal(rcpt, dn)
                    rbps = psD.tile([128, 512], F32, tag="dn")
                    nc.tensor.matmul(rbps, maskt, rcpt, start=True, stop=True)
                    rbt = wkp.tile([128, 512], BF16, tag="rbt")
                    nc.vector.tensor_copy(rbt, rbps)
                    for qg in range(2):
                        nc.vector.tensor_tensor(
                            mhsT[qg][:, NL * b:NL * (b + 1)],
                            mh[:, NL * qg:NL * (qg + 1)],
                            rbt[:, NL * qg:NL * (qg + 1)], AX.mult)

            # ---------- phase 4/5: mix + residual, LN1, FFN ----------
            with tc.tile_pool(name="psB2", bufs=4, space="PSUM") as psB2:
                hid = [pp.tile([128, D], F32, tag=f"hid{tt}", name=f"hid{tt}")
                       for tt in range(4)]
                for tt in range(4):
                    b, s = tt // 2, tt % 2
                    ps = psB2.tile([128, 512], F32, tag="big")
                    for qg in range(2):
                        nc.tensor.matmul(
                            ps[:, 0:D],
                            mhsT[qg][:, NL * b + 128 * s:NL * b + 128 * s + 128],
                            wmixTt[:, D * qg:D * (qg + 1)],
                            start=(qg == 0), stop=(qg == 1))
                    nc.vector.tensor_tensor(hid[tt], ps[:, 0:D], qN[tt], AX.add)
                hrT = pp.tile([128, 2 * TOK], BF16)
                for tt in range(4):
                    hr = lnp.tile([128, D], F32, tag="hr")
                    _ln_apply(nc, lnp, hid[tt], bcast["g1"], bcast["b1"], hr)
                    hrb = wkp.tile([128, D], BF16, tag="hrb")
                    nc.scalar.copy(hrb, hr)
                    for c in range(2):
                        nc.sync.dma_start(
                            out=hrT[:, TOK * c + 128 * tt:TOK * c + 128 * tt + 128],
                            in_=hrb[:, 128 * c:128 * c + 128], transpose=True)
                ffin = [pp.tile([128, TOK], BF16, tag=f"ffin{m}", name=f"ffin{m}")
                        for m in range(8)]
                for m in range(8):
                    ps0 = psB2.tile([128, TOK], F32, tag="big")
                    ps1 = psB2.tile([128, TOK], F32, tag="big")
                    for c in range(2):
                        nc.tensor.matmul(
                            ps0, wi0Tt[:, D4 * c + 128 * m:D4 * c + 128 * m + 128],
                            hrT[:, TOK * c:TOK * (c + 1)],
                            start=(c == 0), stop=(c == 1))
                    for c in range(2):
                        nc.tensor.matmul(
                            ps1, wi1Tt[:, D4 * c + 128 * m:D4 * c + 128 * m + 128],
                            hrT[:, TOK * c:TOK * (c + 1)],
                            start=(c == 0), stop=(c == 1))
                    gt = wkp.tile([128, TOK], BF16, tag="gelu")
                    nc.scalar.activation(gt, ps0, gelu_af)
                    ut = wkp.tile([128, TOK], BF16, tag="u1c")
                    nc.vector.tensor_copy(ut, ps1)
                    nc.vector.tensor_tensor(ffin[m], gt, ut, AX.mult)
                for tt in range(4):
                    b, s = tt // 2, tt % 2
                    ps = psB2.tile([128, 512], F32, tag="big")
                    for m in range(8):
                        nc.tensor.matmul(
                            ps[:, 0:D],
                            ffin[m][:, NL * b + 128 * s:NL * b + 128 * s + 128],
                            woTt[:, D * m:D * (m + 1)],
                            start=(m == 0), stop=(m == 7))
                    o = wkp.tile([128, D], F32, tag="outN")
                    nc.vector.tensor_tensor(o, ps[:, 0:D], hid[tt], AX.add)
                    nc.sync.dma_start(
                        out=out_ext[b].rearrange("(s p) d -> s p d", p=128)[s],
                        in_=o)
    nc.finalize()
    return nc


def prepare_in_maps(inputs):
    bf = ml_dtypes.bfloat16
    X = np.asarray(inputs["X"], np.float32)
    Y = np.asarray(inputs["Y"], np.float32)
    add = np.asarray(inputs["add_enc"], np.float32)
    mult = np.asarray(inputs["mult_enc"], np.float32)
    Ap = add + 16.0 * np.log(mult)                  # [H, Nq, Nk]
    ApT = np.ascontiguousarray(Ap.transpose(0, 2, 1)).astype(bf)   # [H, Nk, Nq]
    RT = np.ascontiguousarray(
        (1.0 / mult).transpose(0, 2, 1)).astype(bf)
    com = {}
    for k in ("Wq", "Wk", "Wv", "Wmix", "wi0", "wi1"):
        W = np.asarray(inputs[k], np.float32)
        com[k + "T"] = np.ascontiguousarray(W.T).reshape(
            W.shape[1] // 128, 128, W.shape[0]).astype(bf)
    wo = np.asarray(inputs["wo"], np.float32)
    com["woT"] = np.ascontiguousarray(wo.T).reshape(8, 128, D).astype(bf)
    com["Yt"] = np.stack([
        np.ascontiguousarray(Y[b].T).reshape(2, 128, N) for b in range(B)
    ]).astype(bf)
    for k in ("bq", "bv", "bmix", "g0", "b0", "g1", "b1"):
        com[k] = np.asarray(inputs[k], np.float32)
    in_maps = []
    for c in range(NCORES):
        sl = slice(c * NL, (c + 1) * NL)
        m = dict(com)
        m["Xs"] = np.ascontiguousarray(X[:, sl, :])
        at = np.empty((B, NKT, 128, 4 * NL), bf)
        rt = np.empty((B, NKT, 128, 4 * NL), bf)
        for b in range(B):
            for j in range(4):
                e = (2 * j + b) % 8
                at[b, :, :, j * NL:(j + 1) * NL] = \
                    ApT[e][:, sl].reshape(NKT, 128, NL)
                rt[b, :, :, j * NL:(j + 1) * NL] = \
                    RT[e][:, sl].reshape(NKT, 128, NL)
        m["addT"] = at
        m["multR"] = rt
        in_maps.append(m)
    return in_maps


def kernel(**inputs):
    in_maps = prepare_in_maps(inputs)
    nc = build_kernel()
    res = run_bass_kernel_spmd(nc, in_maps, list(range(NCORES)))
    out = np.empty((B, N, D), np.float32)
    for c in range(NCORES):
        out[:, c * NL:(c + 1) * NL, :] = res.results[c]["out"]
    return out


if __name__ == "__main__":
    nc = build_kernel()
    print("build OK")


# revision 21
# speedup vs baseline: 2.2444x; 1.1578x over previous
"""Trainium2 Bass kernel for nn_MAB_17471926960685 (dense_transformer).

Token-parallel over N: each core takes a 256-token query slice (both
batches); K/V computed for full N from host-pretransposed Y^T.

Attention dataflow (scores^T: keys on partitions, queries free):
  - Tables host-folded: A' = add_enc + 16*ln(mult_enc)  (numerator table)
                        R  = 1/mult_enc                 (denominator fixup)
    with the reference's head mapping (head h, batch b uses table
    (2h+b)%8; blocks j=0..3 serve heads j and j+4).
  - QK: 4 heads row-packed via tile_position=(32j,0), K=32 each.
  - A' accumulated into score PSUM via identity matmul.
  - One exp (scale=1/16) evacuates [128,1024] PSUM -> P' bf16 = numerator.
  - P = P' * R on VectorE (bf16 2x) = plain exp for the denominator.
  - den: ones-column matmuls col-packed via tile_position=(0,32j).
  - MH: V-slice matmuls col-packed via tile_position=(0,32j).
"""

import math
import sys

import numpy as np
import ml_dtypes

sys.path.insert(0, "/opt/trn_rl_repo")

import concourse.bass as bass
import concourse.mybir as mybir
import concourse.tile as tile
from concourse import bacc
from concourse.masks import make_identity
from concourse.bass_utils import run_bass_kernel_spmd

B, N, D, H = 2, 2048, 256, 8
DS = D // H          # 32
NCORES = 8
NL = N // NCORES     # 256 queries per core per batch
TOK = B * NL         # 512
NKT = N // 128       # 16 key tiles
D4 = 4 * D           # 1024
EPS = 1e-5
F32 = mybir.dt.float32
BF16 = mybir.dt.bfloat16
AX = mybir.AluOpType
AF = mybir.ActivationFunctionType


def _ln_apply(nc, pool, x_ap, g_bc, b_bc, out_ap):
    """LayerNorm rows of x_ap [128, D] -> out_ap (f32)."""
    stats = pool.tile([128, 6], F32, tag="ln_stats")
    mv = pool.tile([128, 2], F32, tag="ln_mv")
    nc.vector.bn_stats(out=stats, in_=x_ap)
    nc.vector.bn_aggr(out=mv, in_=stats)
    eps_t = pool.tile([128, 1], F32, tag="ln_eps")
    nc.vector.memset(eps_t, EPS)
    std = pool.tile([128, 1], F32, tag="ln_std")
    nc.scalar.activation(std, mv[:, 1:2], AF.Sqrt, bias=eps_t)
    rstd = pool.tile([128, 1], F32, tag="ln_rstd")
    nc.vector.reciprocal(rstd, std)
    xn = pool.tile([128, D], F32, tag="ln_xn")
    nc.vector.tensor_scalar(xn, x_ap, mv[:, 0:1], rstd, AX.subtract, AX.mult)
    nc.vector.tensor_tensor(xn, xn, g_bc, AX.mult)
    nc.vector.tensor_tensor(out_ap, xn, b_bc, AX.add)


def build_kernel(gelu_af=AF.Gelu_apprx_tanh):
    nc = bacc.Bacc()
    P = {}
    for name, shape, dt in [
        ("Xs", [B, NL, D], F32),
        ("bq", [D], F32), ("bv", [D], F32), ("bmix", [D], F32),
        ("g0", [D], F32), ("b0", [D], F32), ("g1", [D], F32), ("b1", [D], F32),
        ("Yt", [B, 2, 128, N], BF16),
        ("WqT", [2, 128, D], BF16), ("WkT", [2, 128, D], BF16),
        ("WvT", [2, 128, D], BF16), ("WmixT", [2, 128, D], BF16),
        ("wi0T", [2, 128, D4], BF16), ("wi1T", [2, 128, D4], BF16),
        ("woT", [8, 128, D], BF16),
        ("addT", [B, NKT, 128, 4 * NL], BF16),
        ("multR", [B, NKT, 128, 4 * NL], BF16),
    ]:
        P[name] = nc.declare_dram_parameter(name, shape, dt, isOutput=False)
    out_ext = nc.declare_dram_parameter("out", [B, NL, D], F32, isOutput=True)

    with tile.TileContext(nc) as tc:
        with tc.tile_pool(name="pp", bufs=1) as pp, \
             tc.tile_pool(name="enc", bufs=3) as encp, \
             tc.tile_pool(name="pex", bufs=3) as pxp, \
             tc.tile_pool(name="ln", bufs=2) as lnp, \
             tc.tile_pool(name="wk", bufs=2) as wkp:

            # ---------- constants ----------
            idb = pp.tile([128, 128], BF16)
            make_identity(nc, idb)
            zstat = pp.tile([128, 128], BF16)
            nc.vector.memset(zstat, 0.0)
            zdum = pp.tile([128, 512], BF16)
            nc.vector.memset(zdum, 0.0)
            ones_blk = pp.tile([128, 32], BF16)
            nc.vector.memset(ones_blk, 0.0)
            nc.vector.memset(ones_blk[:, 0:1], 1.0)
            ones_row = pp.tile([1, TOK], F32)
            nc.vector.memset(ones_row, 1.0)
            # maskt row 32j: ones at cols [32j, 32j+32) — rb broadcast lhsT
            maskt = pp.tile([128, 128], F32)
            nc.vector.memset(maskt, 0.0)
            for j in range(4):
                nc.vector.memset(maskt[32 * j:32 * j + 1,
                                       32 * j:32 * j + 32], 1.0)
            # gfill: 1.0 on non-denominator rows (keeps 1/dn finite there)
            gfill = pp.tile([1, 128], F32)
            nc.vector.memset(gfill, 1.0)
            for j in range(4):
                nc.vector.memset(gfill[0:1, 32 * j:32 * j + 1], 0.0)
            brow_bq = pp.tile([1, D], F32)
            nc.sync.dma_start(out=brow_bq,
                              in_=P["bq"][:].rearrange("(o d) -> o d", o=1))
            bcast = {}
            for nm in ("g0", "b0", "g1", "b1", "bmix", "bv"):
                t = pp.tile([128, D], F32, tag=f"bc_{nm}", name=f"bc_{nm}")
                ap = P[nm][:].rearrange("(o d) -> o d", o=1)
                bap = bass.AP(tensor=ap.tensor, offset=ap.offset,
                              ap=[[0, 128], ap.ap[1]])
                nc.sync.dma_start(out=t, in_=bap)
                bcast[nm] = t

            # ---------- weights (host-pretransposed, plain loads) ----------
            def loadw(hnd, nchunk, width, nm):
                t = pp.tile([128, nchunk * width], BF16, tag=nm, name=nm)
                nc.sync.dma_start(
                    out=t.rearrange("p (c d) -> p c d", c=nchunk),
                    in_=hnd[:].rearrange("c p d -> p c d"))
                return t
            wqTt = loadw(P["WqT"], 2, D, "wqTt")
            wkTt = loadw(P["WkT"], 2, D, "wkTt")
            wvTt = loadw(P["WvT"], 2, D, "wvTt")
            wmixTt = loadw(P["WmixT"], 2, D, "wmixTt")
            wi0Tt = loadw(P["wi0T"], 2, D4, "wi0Tt")
            wi1Tt = loadw(P["wi1T"], 2, D4, "wi1Tt")
            woTt = loadw(P["woT"], 8, D, "woTt")
            ytb = []
            for b in range(B):
                t = pp.tile([128, 2 * N], BF16, tag=f"yt{b}", name=f"yt{b}")
                nc.sync.dma_start(
                    out=t.rearrange("p (c n) -> p c n", c=2),
                    in_=P["Yt"][b].rearrange("c p n -> p c n"))
                ytb.append(t)

            # ---------- phase 1: LN0, Xn^T, Q^T, Q_N ----------
            XnT = pp.tile([128, 2 * TOK], BF16)
            with tc.tile_pool(name="psB", bufs=2, space="PSUM") as psB:
                for b in range(B):
                    x_n = wkp.tile([128, 2 * D], F32, tag="xload")
                    nc.sync.dma_start(
                        out=x_n.rearrange("p (s d) -> p s d", s=2),
                        in_=P["Xs"][b].rearrange("(s p) d -> p s d", p=128))
                    for s in range(2):
                        xo = lnp.tile([128, D], F32, tag="xn")
                        _ln_apply(nc, lnp, x_n[:, s * D:(s + 1) * D],
                                  bcast["g0"], bcast["b0"], xo)
                        xb = wkp.tile([128, D], BF16, tag="xnb")
                        nc.scalar.copy(xb, xo)
                        tt = b * 2 + s
                        for c in range(2):
                            nc.sync.dma_start(
                                out=XnT[:, TOK * c + 128 * tt:
                                        TOK * c + 128 * tt + 128],
                                in_=xb[:, 128 * c:128 * c + 128],
                                transpose=True)
                qTq = [pp.tile([128, TOK], BF16, tag=f"qT{qg}", name=f"qT{qg}")
                       for qg in range(2)]
                for qg in range(2):
                    ps = psB.tile([128, 512], F32, tag="big")
                    for c in range(2):
                        nc.tensor.matmul(
                            ps, wqTt[:, D * c + 128 * qg:D * c + 128 * qg + 128],
                            XnT[:, TOK * c:TOK * (c + 1)],
                            start=(c == 0), stop=False)
                    nc.tensor.matmul(ps, brow_bq[0:1, 128 * qg:128 * qg + 128],
                                     ones_row, start=False, stop=True)
                    nc.vector.tensor_copy(qTq[qg], ps)
                qN = [pp.tile([128, D], F32, tag=f"qN{tt}", name=f"qN{tt}")
                      for tt in range(4)]
                for tt in range(4):
                    ps = psB.tile([128, 512], F32, tag="big")
                    for c in range(2):
                        nc.tensor.matmul(
                            ps[:, 0:D],
                            XnT[:, TOK * c + 128 * tt:TOK * c + 128 * tt + 128],
                            wqTt[:, D * c:D * (c + 1)],
                            start=(c == 0), stop=False)
                    nc.tensor.matmul(ps[:, 0:D], ones_row[0:1, 0:128], brow_bq,
                                     start=False, stop=True)
                    # fold bmix into the residual now
                    nc.vector.tensor_tensor(qN[tt], ps[:, 0:D], bcast["bmix"],
                                            AX.add)

                # ---------- phase 2: K^T (quad-major), V_N ----------
                kTq = [pp.tile([128, N], BF16, tag=f"kT{i}", name=f"kT{i}")
                       for i in range(4)]      # index b*2+qg
                for b in range(B):
                    for qg in range(2):
                        for ck in range(4):
                            ps = psB.tile([128, 512], F32, tag="big")
                            for c in range(2):
                                nc.tensor.matmul(
                                    ps,
                                    wkTt[:, D * c + 128 * qg:
                                         D * c + 128 * qg + 128],
                                    ytb[b][:, N * c + 512 * ck:
                                           N * c + 512 * (ck + 1)],
                                    start=(c == 0), stop=(c == 1))
                            nc.vector.tensor_copy(
                                kTq[b * 2 + qg][:, 512 * ck:512 * (ck + 1)], ps)
                vN = [pp.tile([128, NKT * D], BF16, tag=f"vN{b}", name=f"vN{b}")
                      for b in range(B)]
                for b in range(B):
                    for kt in range(NKT):
                        ps = psB.tile([128, 512], F32, tag="big")
                        for c in range(2):
                            nc.tensor.matmul(
                                ps[:, 0:D],
                                ytb[b][:, N * c + 128 * kt:N * c + 128 * kt + 128],
                                wvTt[:, D * c:D * (c + 1)],
                                start=(c == 0), stop=(c == 1))
                        nc.vector.tensor_tensor(
                            vN[b][:, D * kt:D * (kt + 1)], ps[:, 0:D],
                            bcast["bv"], AX.add)

            # ---------- phase 3: attention ----------
            mhsT = [pp.tile([128, TOK], BF16, tag=f"mhsT{qg}", name=f"mhsT{qg}")
                    for qg in range(2)]
            with tc.tile_pool(name="psS", bufs=1, space="PSUM") as psS, \
                 tc.tile_pool(name="psM", bufs=2, space="PSUM") as psM, \
                 tc.tile_pool(name="psD", bufs=2, space="PSUM") as psD:
                # scoreT: 4 banks; head j owns bank j (cols 512j..), qg
                # parity picks the 256-col half (double buffer). Concurrent
                # row-packed QK matmuls thus never share a PSUM bank.
                scoreT = psS.tile([128, 2048], F32)
                sc3 = scoreT.rearrange("pp (j c) -> pp j c", j=4)
                for b in range(B):
                    mh = psM.tile([128, 512], F32, tag="mh")
                    dn = psD.tile([128, 512], F32, tag="dn")
                    # zero-fill once (start=True writes full bank, sets
                    # has_written everywhere) so col-packed accumulation
                    # below can use start=False throughout.
                    nc.tensor.matmul(mh, zstat, zdum, start=True, stop=False,
                                     skip_group_check=True)
                    nc.tensor.matmul(dn, zstat, zdum, start=True, stop=False,
                                     skip_group_check=True)
                    nc.tensor.matmul(dn, gfill, ones_row[0:1, 0:512],
                                     start=False, stop=False,
                                     skip_group_check=True)

                    def emit_denmh(g):
                        kt, qg, pe, pd, last = g
                        for j in range(4):
                            nc.tensor.matmul(
                                dn[32 * j:32 * j + 32, NL * qg:NL * (qg + 1)],
                                ones_blk, pd[:, 256 * j:256 * (j + 1)],
                                start=False, stop=last,
                                tile_position=(0, 32 * j),
                                skip_group_check=True)
                        for j in range(4):
                            nc.tensor.matmul(
                                mh[32 * j:32 * j + 32, NL * qg:NL * (qg + 1)],
                                vN[b][:, D * kt + 128 * qg + 32 * j:
                                      D * kt + 128 * qg + 32 * j + 32],
                                pe[:, 256 * j:256 * (j + 1)],
                                start=False, stop=last,
                                tile_position=(0, 32 * j),
                                skip_group_check=True)

                    pend = None
                    for kt in range(NKT):
                        addc = encp.tile([128, 4 * NL], BF16, tag="addc")
                        nc.sync.dma_start(out=addc, in_=P["addT"][b][kt])
                        rc = encp.tile([128, 4 * NL], BF16, tag="rc")
                        nc.sync.dma_start(out=rc, in_=P["multR"][b][kt])
                        for qg in range(2):
                            # A' tables first: start=True writes the head's
                            # region (sets has_written), QK then accumulates.
                            for j in range(4):
                                nc.tensor.matmul(
                                    scoreT[:, 512 * j + 256 * qg:
                                           512 * j + 256 * (qg + 1)], idb,
                                    addc[:, 256 * j:256 * (j + 1)],
                                    start=True, stop=False,
                                    skip_group_check=True)
                            for j in range(4):
                                nc.tensor.matmul(
                                    scoreT[:, 512 * j + 256 * qg:
                                           512 * j + 256 * (qg + 1)],
                                    kTq[b * 2 + qg][32 * j:32 * j + 32,
                                                    128 * kt:128 * kt + 128],
                                    qTq[qg][32 * j:32 * j + 32,
                                            NL * b:NL * (b + 1)],
                                    start=False, stop=True,
                                    tile_position=(32 * j, 0),
                                    skip_group_check=True)
                            # deferred den/mh of the previous group: keeps
                            # ready PE work behind the exp dependency chain
                            if pend is not None:
                                emit_denmh(pend)
                            pe = pxp.tile([128, 1024], BF16, tag="pe")
                            nc.scalar.activation(
                                pe.rearrange("pp (j c) -> pp j c", j=4),
                                sc3[:, :, 256 * qg:256 * (qg + 1)],
                                AF.Exp, scale=1.0 / 16.0)
                            pd = pxp.tile([128, 1024], BF16, tag="pd")
                            nc.vector.tensor_tensor(pd, pe, rc, AX.mult)
                            pend = (kt, qg, pe, pd,
                                    kt == NKT - 1 and qg == 1)
                    emit_denmh(pend)
                    rcpt = wkp.tile([128, 512], F32, tag="rcpt")
                    nc.vector.reciprocal(rcpt, dn)
                    rbps = psD.tile([128, 512], F32, tag="dn")
                    nc.tensor.matmul(rbps, maskt, rcpt, start=True, stop=True)
                    rbt = wkp.tile([128, 512], BF16, tag="rbt")
                    nc.vector.tensor_copy(rbt, rbps)
                    for qg in range(2):
                        nc.vector.tensor_tensor(
                            mhsT[qg][:, NL * b:NL * (b + 1)],
                            mh[:, NL * qg:NL * (qg + 1)],
                            rbt[:, NL * qg:NL * (qg + 1)], AX.mult)

            # ---------- phase 4/5: mix + residual, LN1, FFN ----------
            with tc.tile_pool(name="psB2", bufs=4, space="PSUM") as psB2:
                hid = [pp.tile([128, D], F32, tag=f"hid{tt}", name=f"hid{tt}")
                       for tt in range(4)]
                for tt in range(4):
                    b, s = tt // 2, tt % 2
                    ps = psB2.tile([128, 512], F32, tag="big")
                    for qg in range(2):
                        nc.tensor.matmul(
                            ps[:, 0:D],
                            mhsT[qg][:, NL * b + 128 * s:NL * b + 128 * s + 128],
                            wmixTt[:, D * qg:D * (qg + 1)],
                            start=(qg == 0), stop=(qg == 1))
                    nc.vector.tensor_tensor(hid[tt], ps[:, 0:D], qN[tt], AX.add)
                hrT = pp.tile([128, 2 * TOK], BF16)
                for tt in range(4):
                    hr = lnp.tile([128, D], F32, tag="hr")
                    _ln_apply(nc, lnp, hid[tt], bcast["g1"], bcast["b1"], hr)
                    hrb = wkp.tile([128, D], BF16, tag="hrb")
                    nc.scalar.copy(hrb, hr)
                    for c in range(2):
                        nc.sync.dma_start(
                            out=hrT[:, TOK * c + 128 * tt:TOK * c + 128 * tt + 128],
                            in_=hrb[:, 128 * c:128 * c + 128], transpose=True)
                ffin = [pp.tile([128, TOK], BF16, tag=f"ffin{m}", name=f"ffin{m}")
                        for m in range(8)]
                for m in range(8):
                    ps0 = psB2.tile([128, TOK], F32, tag="big")
                    ps1 = psB2.tile([128, TOK], F32, tag="big")
                    for c in range(2):
                        nc.tensor.matmul(
                            ps0, wi0Tt[:, D4 * c + 128 * m:D4 * c + 128 * m + 128],
                            hrT[:, TOK * c:TOK * (c + 1)],
                            start=(c == 0), stop=(c == 1))
                    for c in range(2):
                        nc.tensor.matmul(
                            ps1, wi1Tt[:, D4 * c + 128 * m:D4 * c + 128 * m + 128],
                            hrT[:, TOK * c:TOK * (c + 1)],
                            start=(c == 0), stop=(c == 1))
                    gt = wkp.tile([128, TOK], BF16, tag="gelu")
                    nc.scalar.activation(gt, ps0, gelu_af)
                    ut = wkp.tile([128, TOK], BF16, tag="u1c")
                    nc.vector.tensor_copy(ut, ps1)
                    nc.vector.tensor_tensor(ffin[m], gt, ut, AX.mult)
                for tt in range(4):
                    b, s = tt // 2, tt % 2
                    ps = psB2.tile([128, 512], F32, tag="big")
                    for m in range(8):
                        nc.tensor.matmul(
                            ps[:, 0:D],
                            ffin[m][:, NL * b + 128 * s:NL * b + 128 * s + 128],
                            woTt[:, D * m:D * (m + 1)],
                            start=(m == 0), stop=(m == 7))
                    o = wkp.tile([128, D], F32, tag="outN")
                    nc.vector.tensor_tensor(o, ps[:, 0:D], hid[tt], AX.add)
                    nc.sync.dma_start(
                        out=out_ext[b].rearrange("(s p) d -> s p d", p=128)[s],
                        in_=o)
    nc.finalize()
    return nc


def prepare_in_maps(inputs):
    bf = ml_dtypes.bfloat16
    X = np.asarray(inputs["X"], np.float32)
    Y = np.asarray(inputs["Y"], np.float32)
    add = np.asarray(inputs["add_enc"], np.float32)
    mult = np.asarray(inputs["mult_enc"], np.float32)
    Ap = add + 16.0 * np.log(mult)                  # [H, Nq, Nk]
    ApT = np.ascontiguousarray(Ap.transpose(0, 2, 1)).astype(bf)   # [H, Nk, Nq]
    RT = np.ascontiguousarray(
        (1.0 / mult).transpose(0, 2, 1)).astype(bf)
    com = {}
    for k in ("Wq", "Wk", "Wv", "Wmix", "wi0", "wi1"):
        W = np.asarray(inputs[k], np.float32)
        com[k + "T"] = np.ascontiguousarray(W.T).reshape(
            W.shape[1] // 128, 128, W.shape[0]).astype(bf)
    wo = np.asarray(inputs["wo"], np.float32)
    com["woT"] = np.ascontiguousarray(wo.T).reshape(8, 128, D).astype(bf)
    com["Yt"] = np.stack([
        np.ascontiguousarray(Y[b].T).reshape(2, 128, N) for b in range(B)
    ]).astype(bf)
    for k in ("bq", "bv", "bmix", "g0", "b0", "g1", "b1"):
        com[k] = np.asarray(inputs[k], np.float32)
    in_maps = []
    for c in range(NCORES):
        sl = slice(c * NL, (c + 1) * NL)
        m = dict(com)
        m["Xs"] = np.ascontiguousarray(X[:, sl, :])
        at = np.empty((B, NKT, 128, 4 * NL), bf)
        rt = np.empty((B, NKT, 128, 4 * NL), bf)
        for b in range(B):
            for j in range(4):
                e = (2 * j + b) % 8
                at[b, :, :, j * NL:(j + 1) * NL] = \
                    ApT[e][:, sl].reshape(NKT, 128, NL)
                rt[b, :, :, j * NL:(j + 1) * NL] = \
                    RT[e][:, sl].reshape(NKT, 128, NL)
        m["addT"] = at
        m["multR"] = rt
        in_maps.append(m)
    return in_maps


def kernel(**inputs):
    in_maps = prepare_in_maps(inputs)
    nc = build_kernel()
    res = run_bass_kernel_spmd(nc, in_maps, list(range(NCORES)))
    out = np.empty((B, N, D), np.float32)
    for c in range(NCORES):
        out[:, c * NL:(c + 1) * NL, :] = res.results[c]["out"]
    return out


if __name__ == "__main__":
    nc = build_kernel()
    print("build OK")


# revision 23
# speedup vs baseline: 2.2557x; 1.0050x over previous
"""Trainium2 Bass kernel for nn_MAB_17471926960685 (dense_transformer).

Token-parallel over N: each core takes a 256-token query slice (both
batches); K/V computed for full N from host-pretransposed Y^T.

Attention dataflow (scores^T: keys on partitions, queries free):
  - Tables host-folded: A' = add_enc + 16*ln(mult_enc)  (numerator table)
                        R  = 1/mult_enc                 (denominator fixup)
    with the reference's head mapping (head h, batch b uses table
    (2h+b)%8; blocks j=0..3 serve heads j and j+4).
  - QK: 4 heads row-packed via tile_position=(32j,0), K=32 each.
  - A' accumulated into score PSUM via identity matmul.
  - One exp (scale=1/16) evacuates [128,1024] PSUM -> P' bf16 = numerator.
  - P = P' * R on VectorE (bf16 2x) = plain exp for the denominator.
  - den: ones-column matmuls col-packed via tile_position=(0,32j).
  - MH: V-slice matmuls col-packed via tile_position=(0,32j).
"""

import math
import sys

import numpy as np
import ml_dtypes

sys.path.insert(0, "/opt/trn_rl_repo")

import concourse.bass as bass
import concourse.mybir as mybir
import concourse.tile as tile
from concourse import bacc
from concourse.masks import make_identity
from concourse.bass_utils import run_bass_kernel_spmd

B, N, D, H = 2, 2048, 256, 8
DS = D // H          # 32
NCORES = 8
NL = N // NCORES     # 256 queries per core per batch
TOK = B * NL         # 512
NKT = N // 128       # 16 key tiles
D4 = 4 * D           # 1024
EPS = 1e-5
F32 = mybir.dt.float32
BF16 = mybir.dt.bfloat16
AX = mybir.AluOpType
AF = mybir.ActivationFunctionType


def _ln_apply(nc, pool, x_ap, g_bc, b_bc, out_ap):
    """LayerNorm rows of x_ap [128, D] -> out_ap (f32)."""
    stats = pool.tile([128, 6], F32, tag="ln_stats")
    mv = pool.tile([128, 2], F32, tag="ln_mv")
    nc.vector.bn_stats(out=stats, in_=x_ap)
    nc.vector.bn_aggr(out=mv, in_=stats)
    eps_t = pool.tile([128, 1], F32, tag="ln_eps")
    nc.vector.memset(eps_t, EPS)
    std = pool.tile([128, 1], F32, tag="ln_std")
    nc.scalar.activation(std, mv[:, 1:2], AF.Sqrt, bias=eps_t)
    rstd = pool.tile([128, 1], F32, tag="ln_rstd")
    nc.vector.reciprocal(rstd, std)
    xn = pool.tile([128, D], F32, tag="ln_xn")
    nc.vector.tensor_scalar(xn, x_ap, mv[:, 0:1], rstd, AX.subtract, AX.mult)
    nc.vector.tensor_tensor(xn, xn, g_bc, AX.mult)
    nc.vector.tensor_tensor(out_ap, xn, b_bc, AX.add)


def build_kernel(gelu_af=AF.Gelu_apprx_tanh):
    nc = bacc.Bacc()
    P = {}
    for name, shape, dt in [
        ("Xs", [B, NL, D], F32),
        ("bq", [D], F32), ("bv", [D], F32), ("bmix", [D], F32),
        ("g0", [D], F32), ("b0", [D], F32), ("g1", [D], F32), ("b1", [D], F32),
        ("Yt", [B, 2, 128, N], BF16),
        ("WqT", [2, 128, D], BF16), ("WkT", [2, 128, D], BF16),
        ("WvT", [2, 128, D], BF16), ("WmixT", [2, 128, D], BF16),
        ("wi0T", [2, 128, D4], BF16), ("wi1T", [2, 128, D4], BF16),
        ("woT", [8, 128, D], BF16),
        ("addT", [B, NKT, 128, 4 * NL], BF16),
        ("multR", [B, NKT, 128, 4 * NL], BF16),
    ]:
        P[name] = nc.declare_dram_parameter(name, shape, dt, isOutput=False)
    out_ext = nc.declare_dram_parameter("out", [B, NL, D], F32, isOutput=True)

    with tile.TileContext(nc) as tc:
        with tc.tile_pool(name="pp", bufs=1) as pp, \
             tc.tile_pool(name="enc", bufs=3) as encp, \
             tc.tile_pool(name="pex", bufs=3) as pxp, \
             tc.tile_pool(name="ln", bufs=2) as lnp, \
             tc.tile_pool(name="wk", bufs=2) as wkp:

            # ---------- constants ----------
            idb = pp.tile([128, 128], BF16)
            make_identity(nc, idb)
            zstat = pp.tile([128, 128], BF16)
            nc.vector.memset(zstat, 0.0)
            zdum = pp.tile([128, 512], BF16)
            nc.vector.memset(zdum, 0.0)
            ones_blk = pp.tile([128, 32], BF16)
            nc.vector.memset(ones_blk, 0.0)
            nc.vector.memset(ones_blk[:, 0:1], 1.0)
            ones_row = pp.tile([1, TOK], F32)
            nc.vector.memset(ones_row, 1.0)
            # maskt row 32j: ones at cols [32j, 32j+32) — rb broadcast lhsT
            maskt = pp.tile([128, 128], F32)
            nc.vector.memset(maskt, 0.0)
            for j in range(4):
                nc.vector.memset(maskt[32 * j:32 * j + 1,
                                       32 * j:32 * j + 32], 1.0)
            # gfill: 1.0 on non-denominator rows (keeps 1/dn finite there)
            gfill = pp.tile([1, 128], F32)
            nc.vector.memset(gfill, 1.0)
            for j in range(4):
                nc.vector.memset(gfill[0:1, 32 * j:32 * j + 1], 0.0)
            brow_bq = pp.tile([1, D], F32)
            nc.sync.dma_start(out=brow_bq,
                              in_=P["bq"][:].rearrange("(o d) -> o d", o=1))
            bcast = {}
            for nm in ("g0", "b0", "g1", "b1", "bmix", "bv"):
                t = pp.tile([128, D], F32, tag=f"bc_{nm}", name=f"bc_{nm}")
                ap = P[nm][:].rearrange("(o d) -> o d", o=1)
                bap = bass.AP(tensor=ap.tensor, offset=ap.offset,
                              ap=[[0, 128], ap.ap[1]])
                nc.sync.dma_start(out=t, in_=bap)
                bcast[nm] = t

            # ---------- weights (host-pretransposed, plain loads) ----------
            def loadw(hnd, nchunk, width, nm):
                t = pp.tile([128, nchunk * width], BF16, tag=nm, name=nm)
                nc.sync.dma_start(
                    out=t.rearrange("p (c d) -> p c d", c=nchunk),
                    in_=hnd[:].rearrange("c p d -> p c d"))
                return t
            wqTt = loadw(P["WqT"], 2, D, "wqTt")
            wkTt = loadw(P["WkT"], 2, D, "wkTt")
            wvTt = loadw(P["WvT"], 2, D, "wvTt")
            wmixTt = loadw(P["WmixT"], 2, D, "wmixTt")
            wi0Tt = loadw(P["wi0T"], 2, D4, "wi0Tt")
            wi1Tt = loadw(P["wi1T"], 2, D4, "wi1Tt")
            woTt = loadw(P["woT"], 8, D, "woTt")
            ytb = []
            for b in range(B):
                t = pp.tile([128, 2 * N], BF16, tag=f"yt{b}", name=f"yt{b}")
                nc.sync.dma_start(
                    out=t.rearrange("p (c n) -> p c n", c=2),
                    in_=P["Yt"][b].rearrange("c p n -> p c n"))
                ytb.append(t)

            # ---------- phase 1: LN0, Xn^T, Q^T, Q_N ----------
            XnT = pp.tile([128, 2 * TOK], BF16)
            with tc.tile_pool(name="psB", bufs=2, space="PSUM") as psB:
                for b in range(B):
                    x_n = wkp.tile([128, 2 * D], F32, tag="xload")
                    nc.sync.dma_start(
                        out=x_n.rearrange("p (s d) -> p s d", s=2),
                        in_=P["Xs"][b].rearrange("(s p) d -> p s d", p=128))
                    for s in range(2):
                        xo = lnp.tile([128, D], F32, tag="xn")
                        _ln_apply(nc, lnp, x_n[:, s * D:(s + 1) * D],
                                  bcast["g0"], bcast["b0"], xo)
                        xb = wkp.tile([128, D], BF16, tag="xnb")
                        nc.scalar.copy(xb, xo)
                        tt = b * 2 + s
                        for c in range(2):
                            nc.sync.dma_start(
                                out=XnT[:, TOK * c + 128 * tt:
                                        TOK * c + 128 * tt + 128],
                                in_=xb[:, 128 * c:128 * c + 128],
                                transpose=True)
                qTq = [pp.tile([128, TOK], BF16, tag=f"qT{qg}", name=f"qT{qg}")
                       for qg in range(2)]
                for qg in range(2):
                    ps = psB.tile([128, 512], F32, tag="big")
                    for c in range(2):
                        nc.tensor.matmul(
                            ps, wqTt[:, D * c + 128 * qg:D * c + 128 * qg + 128],
                            XnT[:, TOK * c:TOK * (c + 1)],
                            start=(c == 0), stop=False)
                    nc.tensor.matmul(ps, brow_bq[0:1, 128 * qg:128 * qg + 128],
                                     ones_row, start=False, stop=True)
                    nc.vector.tensor_copy(qTq[qg], ps)
                qN = [pp.tile([128, D], F32, tag=f"qN{tt}", name=f"qN{tt}")
                      for tt in range(4)]
                for tt in range(4):
                    ps = psB.tile([128, 512], F32, tag="big")
                    for c in range(2):
                        nc.tensor.matmul(
                            ps[:, 0:D],
                            XnT[:, TOK * c + 128 * tt:TOK * c + 128 * tt + 128],
                            wqTt[:, D * c:D * (c + 1)],
                            start=(c == 0), stop=False)
                    nc.tensor.matmul(ps[:, 0:D], ones_row[0:1, 0:128], brow_bq,
                                     start=False, stop=True)
                    # fold bmix into the residual now
                    nc.vector.tensor_tensor(qN[tt], ps[:, 0:D], bcast["bmix"],
                                            AX.add)

                # ---------- phase 2: K^T (quad-major), V_N ----------
                kTq = [pp.tile([128, N], BF16, tag=f"kT{i}", name=f"kT{i}")
                       for i in range(4)]      # index b*2+qg
                for b in range(B):
                    for qg in range(2):
                        for ck in range(4):
                            ps = psB.tile([128, 512], F32, tag="big")
                            for c in range(2):
                                nc.tensor.matmul(
                                    ps,
                                    wkTt[:, D * c + 128 * qg:
                                         D * c + 128 * qg + 128],
                                    ytb[b][:, N * c + 512 * ck:
                                           N * c + 512 * (ck + 1)],
                                    start=(c == 0), stop=(c == 1))
                            nc.vector.tensor_copy(
                                kTq[b * 2 + qg][:, 512 * ck:512 * (ck + 1)], ps)
                vN = [pp.tile([128, NKT * D], BF16, tag=f"vN{b}", name=f"vN{b}")
                      for b in range(B)]
                for b in range(B):
                    for kt in range(NKT):
                        ps = psB.tile([128, 512], F32, tag="big")
                        for c in range(2):
                            nc.tensor.matmul(
                                ps[:, 0:D],
                                ytb[b][:, N * c + 128 * kt:N * c + 128 * kt + 128],
                                wvTt[:, D * c:D * (c + 1)],
                                start=(c == 0), stop=(c == 1))
                        nc.vector.tensor_tensor(
                            vN[b][:, D * kt:D * (kt + 1)], ps[:, 0:D],
                            bcast["bv"], AX.add)

            # ---------- phase 3: attention ----------
            mhsT = [pp.tile([128, TOK], BF16, tag=f"mhsT{qg}", name=f"mhsT{qg}")
                    for qg in range(2)]
            with tc.tile_pool(name="psS", bufs=1, space="PSUM") as psS, \
                 tc.tile_pool(name="psM", bufs=2, space="PSUM") as psM, \
                 tc.tile_pool(name="psD", bufs=2, space="PSUM") as psD:
                # scoreT: 4 banks; head j owns bank j (cols 512j..), qg
                # parity picks the 256-col half (double buffer). Concurrent
                # row-packed QK matmuls thus never share a PSUM bank.
                scoreT = psS.tile([128, 2048], F32)
                sc3 = scoreT.rearrange("pp (j c) -> pp j c", j=4)
                for b in range(B):
                    mh = psM.tile([128, 512], F32, tag="mh")
                    dn = psD.tile([128, 512], F32, tag="dn")
                    # zero-fill once (start=True writes full bank, sets
                    # has_written everywhere) so col-packed accumulation
                    # below can use start=False throughout.
                    nc.tensor.matmul(mh, zstat, zdum, start=True, stop=False,
                                     skip_group_check=True)
                    nc.tensor.matmul(dn, zstat, zdum, start=True, stop=False,
                                     skip_group_check=True)
                    nc.tensor.matmul(dn, gfill, ones_row[0:1, 0:512],
                                     start=False, stop=False,
                                     skip_group_check=True)

                    def emit_denmh(g):
                        kt, qg, pe, pd, last = g
                        for j in range(4):
                            nc.tensor.matmul(
                                dn[32 * j:32 * j + 32, NL * qg:NL * (qg + 1)],
                                ones_blk, pd[:, 256 * j:256 * (j + 1)],
                                start=False, stop=last,
                                tile_position=(0, 32 * j),
                                skip_group_check=True)
                        for j in range(4):
                            nc.tensor.matmul(
                                mh[32 * j:32 * j + 32, NL * qg:NL * (qg + 1)],
                                vN[b][:, D * kt + 128 * qg + 32 * j:
                                      D * kt + 128 * qg + 32 * j + 32],
                                pe[:, 256 * j:256 * (j + 1)],
                                start=False, stop=last,
                                tile_position=(0, 32 * j),
                                skip_group_check=True)

                    pend = None
                    for kt in range(NKT):
                        addc = encp.tile([128, 4 * NL], BF16, tag="addc")
                        nc.sync.dma_start(out=addc, in_=P["addT"][b][kt])
                        rc = encp.tile([128, 4 * NL], BF16, tag="rc")
                        nc.sync.dma_start(out=rc, in_=P["multR"][b][kt])
                        for qg in range(2):
                            # A' tables first: start=True writes the head's
                            # region (sets has_written), QK then accumulates.
                            for j in range(4):
                                nc.tensor.matmul(
                                    scoreT[:, 512 * j + 256 * qg:
                                           512 * j + 256 * (qg + 1)], idb,
                                    addc[:, 256 * j:256 * (j + 1)],
                                    start=True, stop=False,
                                    skip_group_check=True)
                            for j in range(4):
                                nc.tensor.matmul(
                                    scoreT[:, 512 * j + 256 * qg:
                                           512 * j + 256 * (qg + 1)],
                                    kTq[b * 2 + qg][32 * j:32 * j + 32,
                                                    128 * kt:128 * kt + 128],
                                    qTq[qg][32 * j:32 * j + 32,
                                            NL * b:NL * (b + 1)],
                                    start=False, stop=True,
                                    tile_position=(32 * j, 0),
                                    skip_group_check=True)
                            # deferred den/mh of the previous group: keeps
                            # ready PE work behind the exp dependency chain
                            if pend is not None:
                                emit_denmh(pend)
                            pe = pxp.tile([128, 1024], BF16, tag="pe")
                            nc.scalar.activation(
                                pe.rearrange("pp (j c) -> pp j c", j=4),
                                sc3[:, :, 256 * qg:256 * (qg + 1)],
                                AF.Exp, scale=1.0 / 16.0)
                            pd = pxp.tile([128, 1024], BF16, tag="pd")
                            nc.vector.tensor_tensor(pd, pe, rc, AX.mult)
                            pend = (kt, qg, pe, pd,
                                    kt == NKT - 1 and qg == 1)
                    emit_denmh(pend)
                    rcpt = wkp.tile([128, 512], F32, tag="rcpt")
                    nc.vector.reciprocal(rcpt, dn)
                    rbps = psD.tile([128, 512], F32, tag="dn")
                    nc.tensor.matmul(rbps, maskt, rcpt, start=True, stop=True)
                    rbt = wkp.tile([128, 512], BF16, tag="rbt")
                    nc.vector.tensor_copy(rbt, rbps)
                    for qg in range(2):
                        nc.vector.tensor_tensor(
                            mhsT[qg][:, NL * b:NL * (b + 1)],
                            mh[:, NL * qg:NL * (qg + 1)],
                            rbt[:, NL * qg:NL * (qg + 1)], AX.mult)

            # ---------- phase 4/5: mix + residual, LN1, FFN ----------
            with tc.tile_pool(name="psB2", bufs=4, space="PSUM") as psB2:
                hid = [pp.tile([128, D], F32, tag=f"hid{tt}", name=f"hid{tt}")
                       for tt in range(4)]
                for tt in range(4):
                    b, s = tt // 2, tt % 2
                    ps = psB2.tile([128, 512], F32, tag="big")
                    for qg in range(2):
                        nc.tensor.matmul(
                            ps[:, 0:D],
                            mhsT[qg][:, NL * b + 128 * s:NL * b + 128 * s + 128],
                            wmixTt[:, D * qg:D * (qg + 1)],
                            start=(qg == 0), stop=(qg == 1))
                    nc.vector.tensor_tensor(hid[tt], ps[:, 0:D], qN[tt], AX.add)
                hrT = pp.tile([128, 2 * TOK], BF16)
                for tt in range(4):
                    hr = lnp.tile([128, D], F32, tag="hr")
                    _ln_apply(nc, lnp, hid[tt], bcast["g1"], bcast["b1"], hr)
                    hrb = wkp.tile([128, D], BF16, tag="hrb")
                    nc.scalar.copy(hrb, hr)
                    for c in range(2):
                        nc.sync.dma_start(
                            out=hrT[:, TOK * c + 128 * tt:TOK * c + 128 * tt + 128],
                            in_=hrb[:, 128 * c:128 * c + 128], transpose=True)
                ffin = [pp.tile([128, TOK], BF16, tag=f"ffin{m}", name=f"ffin{m}")
                        for m in range(8)]
                for m in range(8):
                    ps0 = psB2.tile([128, TOK], F32, tag="big")
                    ps1 = psB2.tile([128, TOK], F32, tag="big")
                    for c in range(2):
                        nc.tensor.matmul(
                            ps0, wi0Tt[:, D4 * c + 128 * m:D4 * c + 128 * m + 128],
                            hrT[:, TOK * c:TOK * (c + 1)],
                            start=(c == 0), stop=(c == 1))
                    for c in range(2):
                        nc.tensor.matmul(
                            ps1, wi1Tt[:, D4 * c + 128 * m:D4 * c + 128 * m + 128],
                            hrT[:, TOK * c:TOK * (c + 1)],
                            start=(c == 0), stop=(c == 1))
                    gt = wkp.tile([128, TOK], BF16, tag="gelu")
                    nc.scalar.activation(gt, ps0, gelu_af)
                    ut = wkp.tile([128, TOK], BF16, tag="u1c")
                    nc.vector.tensor_copy(ut, ps1)
                    nc.vector.tensor_tensor(ffin[m], gt, ut, AX.mult)
                for tt in range(4):
                    b, s = tt // 2, tt % 2
                    ps = psB2.tile([128, 512], F32, tag="big")
                    for m in range(8):
                        nc.tensor.matmul(
                            ps[:, 0:D],
                            ffin[m][:, NL * b + 128 * s:NL * b + 128 * s + 128],
                            woTt[:, D * m:D * (m + 1)],
                            start=(m == 0), stop=(m == 7))
                    o = wkp.tile([128, D], F32, tag="outN")
                    nc.vector.tensor_tensor(o, ps[:, 0:D], hid[tt], AX.add)
                    nc.sync.dma_start(
                        out=out_ext[b].rearrange("(s p) d -> s p d", p=128)[s],
                        in_=o)
    nc.finalize()
    return nc


def prepare_in_maps(inputs):
    bf = ml_dtypes.bfloat16
    X = np.asarray(inputs["X"], np.float32)
    Y = np.asarray(inputs["Y"], np.float32)
    add = np.asarray(inputs["add_enc"], np.float32)
    mult = np.asarray(inputs["mult_enc"], np.float32)
    Ap = add + 16.0 * np.log(mult)                  # [H, Nq, Nk]
    ApT = np.ascontiguousarray(Ap.transpose(0, 2, 1)).astype(bf)   # [H, Nk, Nq]
    RT = np.ascontiguousarray(
        (1.0 / mult).transpose(0, 2, 1)).astype(bf)
    com = {}
    for k in ("Wq", "Wk", "Wv", "Wmix", "wi0", "wi1"):
        W = np.asarray(inputs[k], np.float32)
        com[k + "T"] = np.ascontiguousarray(W.T).reshape(
            W.shape[1] // 128, 128, W.shape[0]).astype(bf)
    wo = np.asarray(inputs["wo"], np.float32)
    com["woT"] = np.ascontiguousarray(wo.T).reshape(8, 128, D).astype(bf)
    com["Yt"] = np.stack([
        np.ascontiguousarray(Y[b].T).reshape(2, 128, N) for b in range(B)
    ]).astype(bf)
    for k in ("bq", "bv", "bmix", "g0", "b0", "g1", "b1"):
        com[k] = np.asarray(inputs[k], np.float32)
    in_maps = []
    for c in range(NCORES):
        sl = slice(c * NL, (c + 1) * NL)
        m = dict(com)
        m["Xs"] = np.ascontiguousarray(X[:, sl, :])
        at = np.empty((B, NKT, 128, 4 * NL), bf)
        rt = np.empty((B, NKT, 128, 4 * NL), bf)
        for b in range(B):
            for j in range(4):
                e = (2 * j + b) % 8
                at[b, :, :, j * NL:(j + 1) * NL] = \
                    ApT[e][:, sl].reshape(NKT, 128, NL)
                rt[b, :, :, j * NL:(j + 1) * NL] = \
                    RT[e][:, sl].reshape(NKT, 128, NL)
        m["addT"] = at
        m["multR"] = rt
        in_maps.append(m)
    return in_maps


def kernel(**inputs):
    in_maps = prepare_in_maps(inputs)
    nc = build_kernel()
    res = run_bass_kernel_spmd(nc, in_maps, list(range(NCORES)))
    out = np.empty((B, N, D), np.float32)
    for c in range(NCORES):
        out[:, c * NL:(c + 1) * NL, :] = res.results[c]["out"]
    return out


if __name__ == "__main__":
    nc = build_kernel()
    print("build OK")
